# revision 16
# baseline (speedup 1.0000x reference)
"""CE-Enhanced LSTM Trainium2 kernel.

Data-parallel over batch across 8 NeuronCores (16 sequences/core, weights
replicated, no collectives).  On-chip layout keeps every recurrent tensor
"transposed + packed": SBUF tiles are [128 partitions = hidden-index-within-
128-tile, free = (k_tile j, batch b) -> j*16+b].  Matmuls put the (bf16)
weights in the stationary operand (FWL weight loads) and stream 16-column
state slices; all elementwise work runs on full 128 partitions.

Phase 1 (input GEMM): gxT[t] = (x_t @ Wx + b_gates).T packed, written to DRAM.
Phase 2 (scan): 512-step hardware loop; per step
    gates = gxT[t] + Wh.T @ h      (PSUM pack [128, 512], col order i|f|o|g)
    i,f,o = sigmoid, g = tanh; c = f*c + i*g
    kap-chain: relu(c@W1+b1)@W2 -> q = CHI*DT*(.+b2); broadcast via 1xK matmul
    c += q * (c@evo_W + evo_b)
    u = o * tanh(c);  h = tanh(u@mirror_W + mirror_b);  h /= 1+KAPPA*|h|
Host packs/unpacks all layouts; zeta loss is a 6-element host computation.
"""

import sys

if "/opt/trn_rl_repo" not in sys.path:
    sys.path.insert(0, "/opt/trn_rl_repo")

import numpy as np
import ml_dtypes

import concourse.bass as bass
import concourse.bacc as bacc
import concourse.mybir as mybir
import concourse.tile as tile
from concourse import bass_utils
from concourse.bass_interp import get_hw_module

F32 = mybir.dt.float32
BF16 = mybir.dt.bfloat16
FP8 = mybir.dt.float8e4
AF = mybir.ActivationFunctionType
FP8_SCALE = 256.0  # evo_W / curv_W1 stored as fp8 * 256; folded back via q
USE_FP8 = False
UNROLL = 2
WDT = FP8 if USE_FP8 else BF16
_QS = FP8_SCALE if USE_FP8 else 1.0

B, S, IN, HID = 128, 512, 1024, 1024
CHI, KAPPA, DT = 0.638, 0.35, 0.01
NCORES = 8
BL = B // NCORES            # 16 sequences per core
J = HID // 128              # 8 hidden k-tiles
TOK = BL * S                # 8192 tokens per core
POS2GATE = [0, 1, 3, 2]     # psum pack order i|f|o|g -> gate index in weights

_CACHE = {}


def _build_program(n_steps=S, n_chunks=TOK // 512):
    nc = bacc.Bacc("TRN2", target_bir_lowering=False, debug=False,
                   num_devices=NCORES)

    xT_d = nc.dram_tensor("xT", [IN, TOK], BF16, kind="ExternalInput")
    wx_d = nc.dram_tensor("wx", [IN, 4 * HID], BF16, kind="ExternalInput")
    wh_d = nc.dram_tensor("wh", [HID, 4 * HID], BF16, kind="ExternalInput")
    evo_d = nc.dram_tensor("evo", [HID, HID], WDT, kind="ExternalInput")
    mir_d = nc.dram_tensor("mir", [HID, HID], BF16, kind="ExternalInput")
    cw1_d = nc.dram_tensor("cw1", [HID, HID // 2], WDT, kind="ExternalInput")
    w2_d = nc.dram_tensor("w2", [128, 4], BF16, kind="ExternalInput")
    ones_d = nc.dram_tensor("ones1", [1, 128], BF16, kind="ExternalInput")
    bg_d = nc.dram_tensor("bg", [128, 32], F32, kind="ExternalInput")
    evob_d = nc.dram_tensor("evob", [128, 128], F32, kind="ExternalInput")
    mirb_d = nc.dram_tensor("mirb", [128, 128], F32, kind="ExternalInput")
    c1b_d = nc.dram_tensor("c1b", [128, 64], F32, kind="ExternalInput")
    cb2_d = nc.dram_tensor("cb2", [1, 1], F32, kind="ExternalInput")

    gx_d = nc.dram_tensor("gxT", [S * 128, 512], BF16, kind="Internal")
    ys_d = nc.dram_tensor("ys", [S * 128, 128], F32, kind="ExternalOutput")
    co_d = nc.dram_tensor("cout", [128, 128], F32, kind="ExternalOutput")

    with tile.TileContext(nc) as tc:
        # ---------------- Phase 1: input GEMM ----------------
        with tc.tile_pool(name="gemm_w", bufs=1) as wp, \
             tc.tile_pool(name="gemm_x", bufs=2) as xp, \
             tc.tile_pool(name="gemm_s", bufs=3) as sp, \
             tc.tile_pool(name="gemm_b", bufs=1) as bp, \
             tc.tile_pool(name="gemm_ps", bufs=2, space="PSUM") as pp:
            wx_sb = wp.tile([128, J, 4 * HID], BF16)
            nc.sync.dma_start(
                wx_sb[:], wx_d.ap().rearrange("(k p) n -> p k n", p=128))
            bg_sb = bp.tile([128, 32], F32)
            nc.sync.dma_start(bg_sb[:], bg_d.ap())

            with tc.For_i(0, n_chunks) as ci:
                xc = xp.tile([128, J, 512], BF16, tag="xc")
                nc.sync.dma_start(
                    xc[:],
                    xT_d.ap()[:, bass.ts(ci, 512)]
                        .rearrange("(k p) c -> p k c", p=128))
                # one chunk = 32 steps; buffer all 32 m-tiles so both DMA
                # sides stay contiguous (the m<->t transpose is an on-chip
                # strided DVE copy instead of a 32B-granule DRAM scatter)
                st_all = xp.tile([128, 32, 512], BF16, tag="stall")
                for m in range(32):
                    g, jj = POS2GATE[m // 8], m % 8
                    col0 = g * HID + jj * 128
                    ps = pp.tile([128, 512], F32, tag="gps")
                    for k in range(J):
                        nc.tensor.matmul(
                            ps[:], wx_sb[:, k, col0:col0 + 128], xc[:, k, :],
                            start=(k == 0), stop=(k == J - 1))
                    nc.scalar.activation(st_all[:, m, :], ps[:], AF.Identity,
                                         bias=bg_sb[:, m:m + 1])
                for tt in range(32):
                    stp = sp.tile([128, 512], BF16, tag="stp")
                    nc.vector.tensor_copy(
                        stp[:].rearrange("p (m b) -> p m b", m=32),
                        st_all[:, :, tt * 16:tt * 16 + 16])
                    nc.sync.dma_start(
                        gx_d.ap()[bass.ds(ci * 4096 + tt * 128, 128), :],
                        stp[:])

        # ---------------- Phase 2: recurrent scan ----------------
        with tc.tile_pool(name="scan_w", bufs=1) as wp, \
             tc.tile_pool(name="scan_state", bufs=1) as st_p, \
             tc.tile_pool(name="scan_loop", bufs=2) as lp, \
             tc.tile_pool(name="scan_ps2", bufs=2, space="PSUM") as pp2, \
             tc.tile_pool(name="scan_ps", bufs=1, space="PSUM") as pp:
            wh_sb = wp.tile([128, J, 4 * HID], BF16)
            evo_sb = wp.tile([128, J, HID], WDT)
            mir_sb = wp.tile([128, J, HID], BF16)
            cw1_sb = wp.tile([128, J, HID // 2], WDT)
            w2_sb = wp.tile([128, 4], BF16)
            ones_sb = wp.tile([1, 128], BF16)
            evob_sb = wp.tile([128, 128], F32)
            mirb_sb = wp.tile([128, 128], F32)
            c1b_sb = wp.tile([128, 64], F32)
            cb2_sb = wp.tile([1, 1], F32)
            nc.sync.dma_start(
                wh_sb[:], wh_d.ap().rearrange("(k p) n -> p k n", p=128))
            nc.sync.dma_start(
                evo_sb[:], evo_d.ap().rearrange("(k p) n -> p k n", p=128))
            nc.sync.dma_start(
                mir_sb[:], mir_d.ap().rearrange("(k p) n -> p k n", p=128))
            nc.sync.dma_start(
                cw1_sb[:], cw1_d.ap().rearrange("(k p) n -> p k n", p=128))
            nc.sync.dma_start(w2_sb[:], w2_d.ap())
            nc.sync.dma_start(ones_sb[:], ones_d.ap())
            nc.sync.dma_start(evob_sb[:], evob_d.ap())
            nc.sync.dma_start(mirb_sb[:], mirb_d.ap())
            nc.sync.dma_start(c1b_sb[:], c1b_d.ap())
            nc.sync.dma_start(cb2_sb[:], cb2_d.ap())

            h_bf = st_p.tile([128, 128], BF16)
            c_f = st_p.tile([128, 128], F32)
            nc.vector.memset(h_bf[:], 0.0)
            nc.vector.memset(c_f[:], 0.0)

            def scan_step(i):
                gxt = lp.tile([128, 512], BF16, tag="gxt")
                nc.sync.dma_start(gxt[:], gx_d.ap()[bass.ts(i, 128), :])

                gates_ps = pp2.tile([128, 512], F32, tag="gates")
                for m in range(32):
                    g, jj = POS2GATE[m // 8], m % 8
                    col0 = g * HID + jj * 128
                    for k in range(J):
                        nc.tensor.matmul(
                            gates_ps[:, m * 16:m * 16 + 16],
                            wh_sb[:, k, col0:col0 + 128],
                            h_bf[:, k * 16:k * 16 + 16],
                            start=(k == 0), stop=(k == J - 1))

                gsum = lp.tile([128, 512], F32, tag="gsum")
                nc.vector.tensor_add(gsum[:], gates_ps[:], gxt[:])
                ifog = lp.tile([128, 512], F32, tag="ifog")
                nc.scalar.activation(ifog[:, 0:384], gsum[:, 0:384], AF.Sigmoid)
                nc.scalar.activation(ifog[:, 384:512], gsum[:, 384:512], AF.Tanh)

                ig = lp.tile([128, 128], F32, tag="ig")
                nc.vector.tensor_mul(ig[:], ifog[:, 0:128], ifog[:, 384:512])
                nc.vector.tensor_mul(c_f[:], ifog[:, 128:256], c_f[:])
                nc.vector.tensor_add(c_f[:], c_f[:], ig[:])
                c_f8 = lp.tile([128, 128], WDT, tag="cf8")
                nc.vector.tensor_copy(c_f8[:], c_f[:])

                # curvature matmul 1 (fp8, x FP8_SCALE)
                cur_ps = pp.tile([128, 64], F32, tag="cur")
                for m in range(4):
                    for k in range(J):
                        nc.tensor.matmul(
                            cur_ps[:, m * 16:m * 16 + 16],
                            cw1_sb[:, k, m * 128:m * 128 + 128],
                            c_f8[:, k * 16:k * 16 + 16],
                            start=(k == 0), stop=(k == J - 1))
                # evolution (fp8, x FP8_SCALE) - issued before curv tail so the
                # relu/q chain hides under it
                evo_ps = pp.tile([128, 128], F32, tag="evo")
                for m in range(J):
                    for k in range(J):
                        nc.tensor.matmul(
                            evo_ps[:, m * 16:m * 16 + 16],
                            evo_sb[:, k, m * 128:m * 128 + 128],
                            c_f8[:, k * 16:k * 16 + 16],
                            start=(k == 0), stop=(k == J - 1))
                relu_in = lp.tile([128, 64], F32, tag="rin")
                nc.vector.tensor_add(relu_in[:], cur_ps[:], c1b_sb[:])
                relu_bf = lp.tile([128, 64], BF16, tag="rbf")
                nc.scalar.activation(relu_bf[:], relu_in[:], AF.Relu)
                kap_ps = pp.tile([1, 16], F32, tag="kap")
                for m in range(4):
                    nc.tensor.matmul(
                        kap_ps[:], w2_sb[:, m:m + 1],
                        relu_bf[:, m * 16:m * 16 + 16],
                        start=(m == 0), stop=(m == 3))
                q_bf = lp.tile([1, 16], BF16, tag="qbf")
                nc.scalar.activation(q_bf[:], kap_ps[:], AF.Identity,
                                     bias=cb2_sb[0:1, 0:1],
                                     scale=CHI * DT / (_QS * _QS))
                kbc_ps = pp.tile([128, 128], F32, tag="kbc")
                for jj in range(J):
                    nc.tensor.matmul(
                        kbc_ps[:, jj * 16:jj * 16 + 16],
                        ones_sb[:], q_bf[:], start=True, stop=True)

                ev1 = lp.tile([128, 128], F32, tag="ev1")
                nc.vector.tensor_add(ev1[:], evo_ps[:], evob_sb[:])
                nc.vector.tensor_mul(ev1[:], ev1[:], kbc_ps[:])
                nc.vector.tensor_add(c_f[:], c_f[:], ev1[:])

                # mirror + guardian
                tc_t = lp.tile([128, 128], F32, tag="tct")
                nc.scalar.activation(tc_t[:], c_f[:], AF.Tanh)
                u_bf = lp.tile([128, 128], BF16, tag="ubf")
                nc.vector.tensor_mul(u_bf[:], ifog[:, 256:384], tc_t[:])
                mir_ps = pp.tile([128, 128], F32, tag="mir")
                for m in range(J):
                    for k in range(J):
                        nc.tensor.matmul(
                            mir_ps[:, m * 16:m * 16 + 16],
                            mir_sb[:, k, m * 128:m * 128 + 128],
                            u_bf[:, k * 16:k * 16 + 16],
                            start=(k == 0), stop=(k == J - 1))
                hm = lp.tile([128, 128], F32, tag="hm")
                nc.vector.tensor_add(hm[:], mir_ps[:], mirb_sb[:])
                ht = lp.tile([128, 128], F32, tag="ht")
                nc.scalar.activation(ht[:], hm[:], AF.Tanh)
                den = lp.tile([128, 128], F32, tag="den")
                nc.scalar.activation(den[:], ht[:], AF.Abs, scale=KAPPA)
                nc.vector.tensor_scalar_add(den[:], den[:], 1.0)
                rec = lp.tile([128, 128], F32, tag="rec")
                nc.vector.reciprocal(rec[:], den[:])
                h_f = lp.tile([128, 128], F32, tag="hf")
                nc.vector.tensor_mul(h_f[:], ht[:], rec[:])
                nc.vector.tensor_copy(h_bf[:], h_f[:])
                nc.sync.dma_start(ys_d.ap()[bass.ts(i, 128), :], h_f[:])

            def unrollable_body(iv0, unroll):
                for r in range(unroll):
                    scan_step(iv0 + r)

            tc.For_i_unrolled_general(
                start=0, end=n_steps, step=1,
                unrollable_body=unrollable_body, max_unroll=UNROLL,
                hint_engines=(mybir.EngineType.PE,))

            nc.sync.dma_start(co_d.ap(), c_f[:])

    nc.compile()
    nc.m = get_hw_module(nc.m)
    return nc


def _pack_bias(v, ntile):
    # v[ntile*128] -> [128, ntile*16]: out[p, j*16+b] = v[j*128+p]
    return np.repeat(np.asarray(v, np.float32).reshape(ntile, 128).T,
                     16, axis=1).copy()


def _zeta_loss(zw, zp):
    zw = np.asarray(zw, np.float64)
    zp = np.asarray(zp, np.float64)
    L = 0.2 + 0.1 * np.arange(6, dtype=np.float64)
    s0 = 0.5 - 2.0j
    sw = 1.0 / (1.0 + np.exp(-zw))
    sp = 1.0 / (1.0 + np.exp(-zp))

    def xi(s):
        return np.sum(sw * 0.5 * (np.exp(-s * L) + sp * np.exp(-(1.0 - s) * L)))

    d = xi(s0) - xi(1.0 - s0)
    return np.float32((d * np.conj(d)).real)


def kernel(**inputs):
    x = np.asarray(inputs["x"], np.float32)
    Wg = np.asarray(inputs["W_gates"], np.float32)
    bg = np.asarray(inputs["b_gates"], np.float32)
    mW = np.asarray(inputs["mirror_W"], np.float32)
    mb = np.asarray(inputs["mirror_b"], np.float32)
    c1W = np.asarray(inputs["curv_W1"], np.float32)
    c1b = np.asarray(inputs["curv_b1"], np.float32)
    c2W = np.asarray(inputs["curv_W2"], np.float32)
    c2b = np.asarray(inputs["curv_b2"], np.float32)
    eW = np.asarray(inputs["evo_W"], np.float32)
    eb = np.asarray(inputs["evo_b"], np.float32)

    if "nc" not in _CACHE:
        _CACHE["nc"] = _build_program()
    nc = _CACHE["nc"]

    bf = lambda a: np.asarray(a, dtype=ml_dtypes.bfloat16)
    Wx, Wh = Wg[:IN], Wg[IN:]
    bg_pack = np.empty((128, 32), np.float32)
    for m in range(32):
        bg_pack[:, m] = bg[POS2GATE[m // 8] * HID + (m % 8) * 128:
                           POS2GATE[m // 8] * HID + (m % 8) * 128 + 128]
    _scale = FP8_SCALE if USE_FP8 else 1.0
    _wnp = ml_dtypes.float8_e4m3 if USE_FP8 else ml_dtypes.bfloat16
    f8 = lambda a: np.asarray(np.asarray(a, np.float32) * _scale, dtype=_wnp)
    common = {
        "wx": bf(Wx), "wh": bf(Wh), "evo": f8(eW), "mir": bf(mW),
        "cw1": f8(c1W), "w2": bf(c2W[:, 0].reshape(4, 128).T),
        "ones1": np.ones((1, 128), ml_dtypes.bfloat16),
        "bg": bg_pack,
        "evob": _pack_bias(eb, 8) * _scale,
        "mirb": _pack_bias(mb, 8),
        "c1b": _pack_bias(c1b, 4) * _scale,
        "cb2": np.full((1, 1), CHI * DT * float(c2b[0]) / _scale, np.float32),
    }
    in_maps = []
    for ci in range(NCORES):
        xs = x[ci * BL:(ci + 1) * BL]                   # [16, 512, 1024]
        xT = xs.transpose(2, 1, 0).reshape(IN, TOK)     # [in, t*16+b]
        in_maps.append({**common, "xT": bf(xT)})

    _CACHE["last_in_maps"] = in_maps
    res = bass_utils.run_bass_kernel_spmd(nc, in_maps,
                                          core_ids=list(range(NCORES)))

    outputs = np.empty((B, S, HID), np.float32)
    c_out = np.empty((B, HID), np.float32)
    for ci in range(NCORES):
        ys = res.results[ci]["ys"].reshape(S, 128, 8, 16)
        outputs[ci * BL:(ci + 1) * BL] = (
            ys.transpose(3, 0, 2, 1).reshape(BL, S, HID))
        co = res.results[ci]["cout"].reshape(128, 8, 16)
        c_out[ci * BL:(ci + 1) * BL] = (
            co.transpose(2, 1, 0).reshape(BL, HID))
    h_out = outputs[:, -1, :].copy()

    loss = _zeta_loss(inputs["zeta_w"], inputs["zeta_p"])
    return outputs, h_out, c_out, loss


# revision 21
# speedup vs baseline: 1.6394x; 1.6394x over previous
"""CE-Enhanced LSTM Trainium2 kernel.

Data-parallel over batch across 8 NeuronCores (16 sequences/core, weights
replicated, no collectives).  On-chip layout keeps every recurrent tensor
"transposed + packed": SBUF tiles are [128 partitions = hidden-index-within-
128-tile, free = (k_tile j, batch b) -> j*16+b].  Matmuls put the (bf16)
weights in the stationary operand (FWL weight loads) and stream 16-column
state slices; all elementwise work runs on full 128 partitions.

Phase 1 (input GEMM): gxT[t] = (x_t @ Wx + b_gates).T packed, written to DRAM.
Phase 2 (scan): 512-step hardware loop; per step
    gates = gxT[t] + Wh.T @ h      (PSUM pack [128, 512], col order i|f|o|g)
    i,f,o = sigmoid, g = tanh; c = f*c + i*g
    kap-chain: relu(c@W1+b1)@W2 -> q = CHI*DT*(.+b2); broadcast via 1xK matmul
    c += q * (c@evo_W + evo_b)
    u = o * tanh(c);  h = tanh(u@mirror_W + mirror_b);  h /= 1+KAPPA*|h|
Host packs/unpacks all layouts; zeta loss is a 6-element host computation.
"""

import sys

if "/opt/trn_rl_repo" not in sys.path:
    sys.path.insert(0, "/opt/trn_rl_repo")

import numpy as np
import ml_dtypes

import concourse.bass as bass
import concourse.bacc as bacc
import concourse.mybir as mybir
import concourse.tile as tile
from concourse import bass_utils
from concourse.bass_interp import get_hw_module

F32 = mybir.dt.float32
BF16 = mybir.dt.bfloat16
FP8 = mybir.dt.float8e4
AF = mybir.ActivationFunctionType
FP8_SCALE = 256.0  # evo_W / curv_W1 stored as fp8 * 256; folded back via q
USE_FP8 = False
UNROLL = 2
WDT = FP8 if USE_FP8 else BF16
_QS = FP8_SCALE if USE_FP8 else 1.0

B, S, IN, HID = 128, 512, 1024, 1024
CHI, KAPPA, DT = 0.638, 0.35, 0.01
NCORES = 8
BL = B // NCORES            # 16 sequences per core
J = HID // 128              # 8 hidden k-tiles
TOK = BL * S                # 8192 tokens per core
POS2GATE = [0, 1, 3, 2]     # psum pack order i|f|o|g -> gate index in weights

_CACHE = {}


def _build_program(n_steps=S, n_chunks=TOK // 512):
    nc = bacc.Bacc("TRN2", target_bir_lowering=False, debug=False,
                   num_devices=NCORES)

    xT_d = nc.dram_tensor("xT", [IN, TOK], BF16, kind="ExternalInput")
    wx_d = nc.dram_tensor("wx", [IN, 4 * HID], BF16, kind="ExternalInput")
    wh_d = nc.dram_tensor("wh", [HID, 4 * HID], BF16, kind="ExternalInput")
    evo_d = nc.dram_tensor("evo", [HID, HID], WDT, kind="ExternalInput")
    mir_d = nc.dram_tensor("mir", [HID, HID], BF16, kind="ExternalInput")
    cw1_d = nc.dram_tensor("cw1", [HID, HID // 2], WDT, kind="ExternalInput")
    w2_d = nc.dram_tensor("w2", [128, 4], BF16, kind="ExternalInput")
    ones_d = nc.dram_tensor("ones1", [1, 128], BF16, kind="ExternalInput")
    bg_d = nc.dram_tensor("bg", [128, 32], F32, kind="ExternalInput")
    evob_d = nc.dram_tensor("evob", [128, 128], F32, kind="ExternalInput")
    mirb_d = nc.dram_tensor("mirb", [128, 128], F32, kind="ExternalInput")
    c1b_d = nc.dram_tensor("c1b", [128, 64], F32, kind="ExternalInput")
    cb2_d = nc.dram_tensor("cb2", [1, 1], F32, kind="ExternalInput")

    gx_d = nc.dram_tensor("gxT", [S * 128, 512], BF16, kind="Internal")
    ys_d = nc.dram_tensor("ys", [S * 128, 128], BF16, kind="ExternalOutput")
    co_d = nc.dram_tensor("cout", [128, 128], F32, kind="ExternalOutput")

    with tile.TileContext(nc) as tc:
        # ---------------- Phase 1: input GEMM ----------------
        with tc.tile_pool(name="gemm_w", bufs=1) as wp, \
             tc.tile_pool(name="gemm_x", bufs=2) as xp, \
             tc.tile_pool(name="gemm_s", bufs=3) as sp, \
             tc.tile_pool(name="gemm_b", bufs=1) as bp, \
             tc.tile_pool(name="gemm_ps", bufs=2, space="PSUM") as pp:
            wx_sb = wp.tile([128, J, 4 * HID], BF16)
            nc.sync.dma_start(
                wx_sb[:], wx_d.ap().rearrange("(k p) n -> p k n", p=128))
            bg_sb = bp.tile([128, 32], F32)
            nc.sync.dma_start(bg_sb[:], bg_d.ap())

            with tc.For_i(0, n_chunks) as ci:
                xc = xp.tile([128, J, 512], BF16, tag="xc")
                nc.sync.dma_start(
                    xc[:],
                    xT_d.ap()[:, bass.ts(ci, 512)]
                        .rearrange("(k p) c -> p k c", p=128))
                # one chunk = 32 steps; buffer all 32 m-tiles so both DMA
                # sides stay contiguous (the m<->t transpose is an on-chip
                # strided DVE copy instead of a 32B-granule DRAM scatter)
                st_all = xp.tile([128, 32, 512], BF16, tag="stall")
                for m in range(32):
                    g, jj = POS2GATE[m // 8], m % 8
                    col0 = g * HID + jj * 128
                    ps = pp.tile([128, 512], F32, tag="gps")
                    for k in range(J):
                        nc.tensor.matmul(
                            ps[:], wx_sb[:, k, col0:col0 + 128], xc[:, k, :],
                            start=(k == 0), stop=(k == J - 1))
                    nc.scalar.activation(st_all[:, m, :], ps[:], AF.Identity,
                                         bias=bg_sb[:, m:m + 1])
                for tt in range(32):
                    stp = sp.tile([128, 512], BF16, tag="stp")
                    nc.vector.tensor_copy(
                        stp[:].rearrange("p (m b) -> p m b", m=32),
                        st_all[:, :, tt * 16:tt * 16 + 16])
                    nc.sync.dma_start(
                        gx_d.ap()[bass.ds(ci * 4096 + tt * 128, 128), :],
                        stp[:])

        # ---------------- Phase 2: recurrent scan ----------------
        with tc.tile_pool(name="scan_w", bufs=1) as wp, \
             tc.tile_pool(name="scan_state", bufs=1) as st_p, \
             tc.tile_pool(name="scan_loop", bufs=2) as lp, \
             tc.tile_pool(name="scan_ps2", bufs=2, space="PSUM") as pp2, \
             tc.tile_pool(name="scan_ps", bufs=1, space="PSUM") as pp:
            wh_sb = wp.tile([128, J, 4 * HID], BF16)
            evo_sb = wp.tile([128, J, HID], WDT)
            mir_sb = wp.tile([128, J, HID], BF16)
            cw1_sb = wp.tile([128, J, HID // 2], WDT)
            w2_sb = wp.tile([128, 4], BF16)
            ones_sb = wp.tile([1, 128], BF16)
            evob_sb = wp.tile([128, 128], F32)
            mirb_sb = wp.tile([128, 128], F32)
            c1b_sb = wp.tile([128, 64], F32)
            cb2_sb = wp.tile([1, 1], F32)
            nc.sync.dma_start(
                wh_sb[:], wh_d.ap().rearrange("(k p) n -> p k n", p=128))
            nc.sync.dma_start(
                evo_sb[:], evo_d.ap().rearrange("(k p) n -> p k n", p=128))
            nc.sync.dma_start(
                mir_sb[:], mir_d.ap().rearrange("(k p) n -> p k n", p=128))
            nc.sync.dma_start(
                cw1_sb[:], cw1_d.ap().rearrange("(k p) n -> p k n", p=128))
            nc.sync.dma_start(w2_sb[:], w2_d.ap())
            nc.sync.dma_start(ones_sb[:], ones_d.ap())
            nc.sync.dma_start(evob_sb[:], evob_d.ap())
            nc.sync.dma_start(mirb_sb[:], mirb_d.ap())
            nc.sync.dma_start(c1b_sb[:], c1b_d.ap())
            nc.sync.dma_start(cb2_sb[:], cb2_d.ap())

            h_bf = st_p.tile([128, 128], BF16)
            c_f = st_p.tile([128, 128], F32)
            nc.vector.memset(h_bf[:], 0.0)
            nc.vector.memset(c_f[:], 0.0)

            def scan_step(i):
                gxt = lp.tile([128, 512], BF16, tag="gxt")
                nc.sync.dma_start(gxt[:], gx_d.ap()[bass.ts(i, 128), :])

                gates_ps = pp2.tile([128, 512], F32, tag="gates")
                for m in range(32):
                    g, jj = POS2GATE[m // 8], m % 8
                    col0 = g * HID + jj * 128
                    for k in range(J):
                        nc.tensor.matmul(
                            gates_ps[:, m * 16:m * 16 + 16],
                            wh_sb[:, k, col0:col0 + 128],
                            h_bf[:, k * 16:k * 16 + 16],
                            start=(k == 0), stop=(k == J - 1))

                # gate epilogue split by gate so the cell chain (and with it
                # the curv/evo matmuls) starts as early as possible; the
                # o-gate sigmoid is deferred past the cast (first needed for u)
                gs_g = lp.tile([128, 128], F32, tag="gsg")
                nc.vector.tensor_add(gs_g[:], gates_ps[:, 384:512],
                                     gxt[:, 384:512])
                g_t = lp.tile([128, 128], F32, tag="gt")
                nc.scalar.activation(g_t[:], gs_g[:], AF.Tanh)
                gs_if = lp.tile([128, 256], F32, tag="gsif")
                nc.vector.tensor_add(gs_if[:], gates_ps[:, 0:256],
                                     gxt[:, 0:256])
                if_t = lp.tile([128, 256], F32, tag="ift")
                nc.scalar.activation(if_t[:], gs_if[:], AF.Sigmoid)

                ig = lp.tile([128, 128], F32, tag="ig")
                nc.vector.tensor_mul(ig[:], if_t[:, 0:128], g_t[:])
                nc.vector.tensor_mul(c_f[:], if_t[:, 128:256], c_f[:])
                nc.vector.tensor_add(c_f[:], c_f[:], ig[:])
                c_f8 = lp.tile([128, 128], WDT, tag="cf8")
                nc.vector.tensor_copy(c_f8[:], c_f[:])

                gs_o = lp.tile([128, 128], F32, tag="gso")
                nc.vector.tensor_add(gs_o[:], gates_ps[:, 256:384],
                                     gxt[:, 256:384])
                o_t = lp.tile([128, 128], F32, tag="ot")
                nc.scalar.activation(o_t[:], gs_o[:], AF.Sigmoid)

                # curvature matmul 1 (fp8, x FP8_SCALE)
                cur_ps = pp.tile([128, 64], F32, tag="cur")
                for m in range(4):
                    for k in range(J):
                        nc.tensor.matmul(
                            cur_ps[:, m * 16:m * 16 + 16],
                            cw1_sb[:, k, m * 128:m * 128 + 128],
                            c_f8[:, k * 16:k * 16 + 16],
                            start=(k == 0), stop=(k == J - 1))
                # evolution (fp8, x FP8_SCALE) - issued before curv tail so the
                # relu/q chain hides under it
                evo_ps = pp.tile([128, 128], F32, tag="evo")
                for m in range(J):
                    for k in range(J):
                        nc.tensor.matmul(
                            evo_ps[:, m * 16:m * 16 + 16],
                            evo_sb[:, k, m * 128:m * 128 + 128],
                            c_f8[:, k * 16:k * 16 + 16],
                            start=(k == 0), stop=(k == J - 1))
                relu_in = lp.tile([128, 64], F32, tag="rin")
                nc.vector.tensor_add(relu_in[:], cur_ps[:], c1b_sb[:])
                relu_bf = lp.tile([128, 64], BF16, tag="rbf")
                nc.scalar.activation(relu_bf[:], relu_in[:], AF.Relu)
                kap_ps = pp.tile([1, 16], F32, tag="kap")
                for m in range(4):
                    nc.tensor.matmul(
                        kap_ps[:], w2_sb[:, m:m + 1],
                        relu_bf[:, m * 16:m * 16 + 16],
                        start=(m == 0), stop=(m == 3))
                q_bf = lp.tile([1, 16], BF16, tag="qbf")
                nc.scalar.activation(q_bf[:], kap_ps[:], AF.Identity,
                                     bias=cb2_sb[0:1, 0:1],
                                     scale=CHI * DT / (_QS * _QS))
                kbc_ps = pp.tile([128, 128], F32, tag="kbc")
                for jj in range(J):
                    nc.tensor.matmul(
                        kbc_ps[:, jj * 16:jj * 16 + 16],
                        ones_sb[:], q_bf[:], start=True, stop=True)

                ev1 = lp.tile([128, 128], F32, tag="ev1")
                nc.vector.tensor_add(ev1[:], evo_ps[:], evob_sb[:])
                nc.vector.tensor_mul(ev1[:], ev1[:], kbc_ps[:])
                nc.vector.tensor_add(c_f[:], c_f[:], ev1[:])

                # mirror + guardian
                tc_t = lp.tile([128, 128], F32, tag="tct")
                nc.scalar.activation(tc_t[:], c_f[:], AF.Tanh)
                u_bf = lp.tile([128, 128], BF16, tag="ubf")
                nc.vector.tensor_mul(u_bf[:], o_t[:], tc_t[:])
                mir_ps = pp.tile([128, 128], F32, tag="mir")
                for m in range(J):
                    for k in range(J):
                        nc.tensor.matmul(
                            mir_ps[:, m * 16:m * 16 + 16],
                            mir_sb[:, k, m * 128:m * 128 + 128],
                            u_bf[:, k * 16:k * 16 + 16],
                            start=(k == 0), stop=(k == J - 1))
                hm = lp.tile([128, 128], F32, tag="hm")
                nc.vector.tensor_add(hm[:], mir_ps[:], mirb_sb[:])
                ht = lp.tile([128, 128], F32, tag="ht")
                nc.scalar.activation(ht[:], hm[:], AF.Tanh)
                den = lp.tile([128, 128], F32, tag="den")
                nc.scalar.activation(den[:], ht[:], AF.Abs, scale=KAPPA)
                nc.vector.tensor_scalar_add(den[:], den[:], 1.0)
                rec = lp.tile([128, 128], F32, tag="rec")
                nc.vector.reciprocal(rec[:], den[:])
                # h state is bf16 anyway; write it directly and emit ys in
                # bf16 too (host upcasts) - shortens the h critical path
                nc.vector.tensor_mul(h_bf[:], ht[:], rec[:])
                nc.sync.dma_start(ys_d.ap()[bass.ts(i, 128), :], h_bf[:])

            def unrollable_body(iv0, unroll):
                for r in range(unroll):
                    scan_step(iv0 + r)

            tc.For_i_unrolled_general(
                start=0, end=n_steps, step=1,
                unrollable_body=unrollable_body, max_unroll=UNROLL,
                hint_engines=(mybir.EngineType.PE,))

            nc.sync.dma_start(co_d.ap(), c_f[:])

    nc.compile()
    nc.m = get_hw_module(nc.m)
    return nc


def _pack_bias(v, ntile):
    # v[ntile*128] -> [128, ntile*16]: out[p, j*16+b] = v[j*128+p]
    return np.repeat(np.asarray(v, np.float32).reshape(ntile, 128).T,
                     16, axis=1).copy()


def _zeta_loss(zw, zp):
    zw = np.asarray(zw, np.float64)
    zp = np.asarray(zp, np.float64)
    L = 0.2 + 0.1 * np.arange(6, dtype=np.float64)
    s0 = 0.5 - 2.0j
    sw = 1.0 / (1.0 + np.exp(-zw))
    sp = 1.0 / (1.0 + np.exp(-zp))

    def xi(s):
        return np.sum(sw * 0.5 * (np.exp(-s * L) + sp * np.exp(-(1.0 - s) * L)))

    d = xi(s0) - xi(1.0 - s0)
    return np.float32((d * np.conj(d)).real)


def kernel(**inputs):
    x = np.asarray(inputs["x"], np.float32)
    Wg = np.asarray(inputs["W_gates"], np.float32)
    bg = np.asarray(inputs["b_gates"], np.float32)
    mW = np.asarray(inputs["mirror_W"], np.float32)
    mb = np.asarray(inputs["mirror_b"], np.float32)
    c1W = np.asarray(inputs["curv_W1"], np.float32)
    c1b = np.asarray(inputs["curv_b1"], np.float32)
    c2W = np.asarray(inputs["curv_W2"], np.float32)
    c2b = np.asarray(inputs["curv_b2"], np.float32)
    eW = np.asarray(inputs["evo_W"], np.float32)
    eb = np.asarray(inputs["evo_b"], np.float32)

    if "nc" not in _CACHE:
        _CACHE["nc"] = _build_program()
    nc = _CACHE["nc"]

    bf = lambda a: np.asarray(a, dtype=ml_dtypes.bfloat16)
    Wx, Wh = Wg[:IN], Wg[IN:]
    bg_pack = np.empty((128, 32), np.float32)
    for m in range(32):
        bg_pack[:, m] = bg[POS2GATE[m // 8] * HID + (m % 8) * 128:
                           POS2GATE[m // 8] * HID + (m % 8) * 128 + 128]
    _scale = FP8_SCALE if USE_FP8 else 1.0
    _wnp = ml_dtypes.float8_e4m3 if USE_FP8 else ml_dtypes.bfloat16
    f8 = lambda a: np.asarray(np.asarray(a, np.float32) * _scale, dtype=_wnp)
    common = {
        "wx": bf(Wx), "wh": bf(Wh), "evo": f8(eW), "mir": bf(mW),
        "cw1": f8(c1W), "w2": bf(c2W[:, 0].reshape(4, 128).T),
        "ones1": np.ones((1, 128), ml_dtypes.bfloat16),
        "bg": bg_pack,
        "evob": _pack_bias(eb, 8) * _scale,
        "mirb": _pack_bias(mb, 8),
        "c1b": _pack_bias(c1b, 4) * _scale,
        "cb2": np.full((1, 1), CHI * DT * float(c2b[0]) / _scale, np.float32),
    }
    in_maps = []
    for ci in range(NCORES):
        xs = x[ci * BL:(ci + 1) * BL]                   # [16, 512, 1024]
        xT = xs.transpose(2, 1, 0).reshape(IN, TOK)     # [in, t*16+b]
        in_maps.append({**common, "xT": bf(xT)})

    _CACHE["last_in_maps"] = in_maps
    res = bass_utils.run_bass_kernel_spmd(nc, in_maps,
                                          core_ids=list(range(NCORES)))

    outputs = np.empty((B, S, HID), np.float32)
    c_out = np.empty((B, HID), np.float32)
    for ci in range(NCORES):
        ys = np.asarray(res.results[ci]["ys"],
                        np.float32).reshape(S, 128, 8, 16)
        outputs[ci * BL:(ci + 1) * BL] = (
            ys.transpose(3, 0, 2, 1).reshape(BL, S, HID))
        co = res.results[ci]["cout"].reshape(128, 8, 16)
        c_out[ci * BL:(ci + 1) * BL] = (
            co.transpose(2, 1, 0).reshape(BL, HID))
    h_out = outputs[:, -1, :].copy()

    loss = _zeta_loss(inputs["zeta_w"], inputs["zeta_p"])
    return outputs, h_out, c_out, loss
